# revision 4
# baseline (speedup 1.0000x reference)
"""Multi-head attention (B=4, S=2048, D=1024, H=16, DK=64) on 8 Trainium2
NeuronCores — v2: PE-array packing via tile_position.

Sharding: core c = 2*b + j handles batch b = c//2 and query rows
[j*1024, (j+1)*1024).  Fully local, no collectives.

v2 changes vs baseline:
  * scores matmuls (K=64) issued as adjacent row-tile pairs (T0 rows
    0-63 / T8 rows 64-127) -> 2 heads' scores run concurrently in the
    PE array: 512 -> 256 effective matmul slots.
  * PV matmuls drop the ones column (M=64) and are col-tiled (T0 cols
    0-63 / T1 cols 64-127) into one cx bank: 512 -> 256 slots.
  * softmax denominators via 4-way col-tiled M=1 ones-matmuls
    (partitions 0/32/64/96 of one PSUM bank): +128 slots.
  * K/Q projection chains restructured to 1-bank [128,512] sub-chains
    so the whole steady state fits in 8 PSUM banks:
    sp ping-pong (4) + cx q0/q1 (2) + den (1) + filler (1).
Layouts otherwise as baseline: X^T resident bf16, per-(head,chunk)
V [128,64], E^T = exp(scores^T/8 + mask - 3) streamed, ctx^T in SBUF,
out = ctx^T-as-lhsT @ Wo.  radd = bv@Wo + bo folded on host.
"""

import numpy as np
import ml_dtypes

B, S, D, H, DK = 4, 2048, 1024, 16, 64
SQ = S // 2          # query rows per core
N_CORES = 8
NEG_C = -3.0         # exp stabilizer; cancels exactly in normalization
BF = ml_dtypes.bfloat16


def _build():
    import concourse.mybir as mybir
    import concourse.tile as tile
    from concourse import bacc

    dt = mybir.dt
    AF = mybir.ActivationFunctionType
    nc = bacc.Bacc("TRN2", num_devices=N_CORES)

    xt = nc.declare_dram_parameter("xt", [D, S], dt.bfloat16, isOutput=False)
    xqt = nc.declare_dram_parameter("xqt", [D, SQ], dt.bfloat16, isOutput=False)
    wq = nc.declare_dram_parameter("wq", [D, D], dt.bfloat16, isOutput=False)
    wk = nc.declare_dram_parameter("wk", [D, D], dt.bfloat16, isOutput=False)
    wv = nc.declare_dram_parameter("wv", [D, D], dt.bfloat16, isOutput=False)
    wo = nc.declare_dram_parameter("wo", [D, D], dt.bfloat16, isOutput=False)
    bq = nc.declare_dram_parameter("bq", [D], dt.float32, isOutput=False)
    bk = nc.declare_dram_parameter("bk", [D], dt.float32, isOutput=False)
    radd = nc.declare_dram_parameter("radd", [D], dt.float32, isOutput=False)
    mk = nc.declare_dram_parameter("mk", [S], dt.float32, isOutput=False)
    out = nc.declare_dram_parameter("out", [SQ, D], dt.float32, isOutput=True)

    with tile.TileContext(nc) as tc:
        with (
            tc.tile_pool(name="pers", bufs=1) as pers,
            tc.tile_pool(name="spp", bufs=2, space="PSUM") as spp,
            tc.tile_pool(name="cxp", bufs=2, space="PSUM") as cxp,
            tc.tile_pool(name="denp", bufs=1, space="PSUM") as denp,
            tc.tile_pool(name="fillp", bufs=1, space="PSUM") as fillp,
            tc.tile_pool(name="ktp", bufs=3) as ktp,
            tc.tile_pool(name="qtp", bufs=3) as qtp,
            tc.tile_pool(name="wstr", bufs=2) as wstr,
        ):
            # ---- persistent SBUF arrays -------------------------------
            v_s = pers.tile([128, 16 * H * DK], dt.bfloat16, tag="v")
            xt_s = pers.tile([128, 8 * S], dt.bfloat16, tag="xt")
            wk_s = pers.tile([128, 8 * D], dt.bfloat16, tag="wk")
            xqt_s = pers.tile([128, 8 * SQ], dt.bfloat16, tag="xqts")
            bqc = pers.tile([128, 8], dt.float32, tag="bqc")
            bkc = pers.tile([128, 8], dt.float32, tag="bkc")
            mkc = pers.tile([128, 16], dt.float32, tag="mkc")
            onesm = pers.tile([128, 32], dt.bfloat16, tag="ones")

            nc.sync.dma_start(out=bqc, in_=bq.rearrange("(a p) -> p a", p=128))
            nc.sync.dma_start(out=bkc, in_=bk.rearrange("(a p) -> p a", p=128))
            nc.sync.dma_start(out=mkc, in_=mk.rearrange("(a p) -> p a", p=128))
            nc.gpsimd.memset(onesm, 1.0)

            kt_tiles = {}
            qt_tiles = {}

            # Filler-unit generators: emit projection chains for pair
            # `i` in small steps so they interleave with attention.
            # Sub-chains use a single PSUM bank ([128,512]).
            def k_chain_units(i):
                kt_t = ktp.tile([128, S], dt.bfloat16, tag="ktt",
                                name=f"ktt{i}")
                kt_tiles[i] = kt_t
                for sf in range(4):
                    pk = fillp.tile([128, 512], dt.float32, tag="fill",
                                    name=f"pk{i}_{sf}")
                    for c in range(8):
                        def do_k(c=c, sf=sf, pk=pk):
                            nc.tensor.matmul(
                                out=pk,
                                lhsT=wk_s[:, c * D + i * 128:
                                          c * D + (i + 1) * 128],
                                rhs=xt_s[:, c * S + sf * 512:
                                         c * S + (sf + 1) * 512],
                                start=(c == 0), stop=(c == 7))
                        yield do_k
                    def drain_k(sf=sf, pk=pk, kt_t=kt_t):
                        nc.vector.tensor_scalar_add(
                            kt_t[:, sf * 512:(sf + 1) * 512],
                            pk, bkc[:, i:i + 1])
                    yield drain_k

            def q_chain_units(i):
                qt_t = qtp.tile([128, SQ], dt.bfloat16, tag="qtt",
                                name=f"qtt{i}")
                qt_tiles[i] = qt_t
                wqc = wstr.tile([128, 1024], dt.bfloat16, tag="wqs",
                                name=f"wqc{i}")
                nc.sync.dma_start(
                    out=wqc.rearrange("p (c n) -> p c n", n=128),
                    in_=wq.rearrange("(c p) n -> p c n", p=128)[
                        :, :, i * 128:(i + 1) * 128])
                for sf in range(2):
                    pq = fillp.tile([128, 512], dt.float32, tag="fill",
                                    name=f"pq{i}_{sf}")
                    for c in range(8):
                        def do_q(c=c, sf=sf, pq=pq, wqc=wqc):
                            nc.tensor.matmul(
                                out=pq,
                                lhsT=wqc[:, c * 128:(c + 1) * 128],
                                rhs=xqt_s[:, c * SQ + sf * 512:
                                          c * SQ + (sf + 1) * 512],
                                start=(c == 0), stop=(c == 7))
                        yield do_q
                    def drain_q(sf=sf, pq=pq, qt_t=qt_t):
                        nc.vector.tensor_scalar_add(
                            qt_t[:, sf * 512:(sf + 1) * 512],
                            pq, bqc[:, i:i + 1])
                    yield drain_q

            def chain(*gens):
                for g in gens:
                    if g is None:
                        continue
                    yield from g

            def drive(gen, n=1):
                if gen is None:
                    return
                for _ in range(n):
                    for u in gen:
                        u()
                        break
                    else:
                        return

            def finish(gen):
                if gen is not None:
                    for u in gen:
                        u()

            # ---- phase 1: V projection (wv freed after) ---------------
            with tc.tile_pool(name="poolA", bufs=1) as poolA:
                wv_s = poolA.tile([128, 8 * D], dt.bfloat16, tag="wv")
                for c in range(8):
                    nc.sync.dma_start(
                        out=xt_s[:, c * S:(c + 1) * S],
                        in_=xt[c * 128:(c + 1) * 128, :])
                    nc.sync.dma_start(
                        out=wv_s[:, c * D:(c + 1) * D],
                        in_=wv[c * 128:(c + 1) * 128, :])
                for c in range(8):
                    nc.sync.dma_start(
                        out=wk_s[:, c * D:(c + 1) * D],
                        in_=wk[c * 128:(c + 1) * 128, :])
                    nc.sync.dma_start(
                        out=xqt_s[:, c * SQ:(c + 1) * SQ],
                        in_=xqt[c * 128:(c + 1) * 128, :])

                # prime pair 0 first: its kt/qt drains overlap V compute
                finish(k_chain_units(0))
                finish(q_chain_units(0))

                for sc in range(16):
                    pv = spp.tile([128, 1024], dt.float32, tag="sp",
                                  name=f"pv{sc}")
                    for c in range(8):
                        lhsT = xt_s[:, c * S + sc * 128: c * S + (sc + 1) * 128]
                        for dv2 in range(2):
                            nc.tensor.matmul(
                                out=pv[:, dv2 * 512:(dv2 + 1) * 512],
                                lhsT=lhsT,
                                rhs=wv_s[:, c * D + dv2 * 512: c * D + (dv2 + 1) * 512],
                                start=(c == 0), stop=(c == 7))
                    # pv cols = 16 heads x 64 dims for this seq chunk
                    nc.vector.tensor_copy(
                        v_s[:, sc * H * DK:(sc + 1) * H * DK], pv)



            # ---- phase 2: interleaved projections + attention ---------
            with (
                tc.tile_pool(name="attin", bufs=1) as attin,
                tc.tile_pool(name="epool", bufs=8) as epool,
                tc.tile_pool(name="rpool", bufs=1) as rpool,
                tc.tile_pool(name="stg", bufs=1) as stg,
                tc.tile_pool(name="opool", bufs=2) as opool,
            ):
                ctxt_s = attin.tile([128, 8 * SQ], dt.bfloat16, tag="ctxt")
                wo_s = attin.tile([128, 8 * D], dt.bfloat16, tag="wo")
                bob = attin.tile([128, D], dt.float32, tag="bob")
                for c in range(8):
                    nc.sync.dma_start(
                        out=wo_s[:, c * D:(c + 1) * D],
                        in_=wo[c * 128:(c + 1) * 128, :])

                def _bcast_src(ap):
                    import concourse.bass as bass
                    return bass.AP(
                        tensor=ap.tensor, offset=ap.offset,
                        ap=[[0, 128]] + [list(p) for p in ap.ap])

                nc.gpsimd.dma_start(out=bob, in_=_bcast_src(radd[:]))

                for i in range(8):
                    he, ho = 2 * i, 2 * i + 1
                    filler = chain(
                        k_chain_units(i + 1) if i + 1 < 8 else None,
                        q_chain_units(i + 1) if i + 1 < 8 else None)
                    kt_t, qt_t = kt_tiles[i], qt_tiles[i]
                    cxq = [cxp.tile([128, 512], dt.float32, tag="cx",
                                    name=f"cx{i}_{q2}") for q2 in range(2)]
                    den = denp.tile([128, 512], dt.float32, tag="den",
                                    name=f"den{i}")
                    e_hist = {}
                    for g in range(17):
                        if g < 16:
                            scc = g
                            for q2 in range(2):
                                sp = spp.tile([128, 1024], dt.float32,
                                              tag="sp", name=f"sp{i}_{scc}_{q2}")
                                # row-tiled pair: T0 (rows 0-63) + T8
                                # (rows 64-127), adjacent -> concurrent
                                nc.tensor.matmul(
                                    out=sp[:, 0:512],
                                    lhsT=kt_t[0:64, scc * 128:(scc + 1) * 128],
                                    rhs=qt_t[0:64, q2 * 512:(q2 + 1) * 512],
                                    start=True, stop=True)
                                nc.tensor.matmul(
                                    out=sp[:, 512:1024],
                                    lhsT=kt_t[64:128, scc * 128:(scc + 1) * 128],
                                    rhs=qt_t[64:128, q2 * 512:(q2 + 1) * 512],
                                    start=True, stop=True)
                                e = epool.tile([128, 1024], dt.bfloat16,
                                               tag="e", name=f"e{i}_{scc}_{q2}")
                                nc.scalar.activation(
                                    out=e, in_=sp, func=AF.Exp,
                                    bias=mkc[:, scc:scc + 1],
                                    scale=1.0 / np.sqrt(DK))
                                e_hist[(scc, q2)] = e
                            drive(filler, 2)
                        if g >= 2 and g % 2 == 0:
                            for scc in (g - 2, g - 1):
                                st = (scc == 0)
                                sp_ = (scc == 15)
                                eA = e_hist[(scc, 0)]
                                eB = e_hist[(scc, 1)]
                                vhe = v_s[:, (scc * H + he) * DK:
                                          (scc * H + he + 1) * DK]
                                vho = v_s[:, (scc * H + ho) * DK:
                                          (scc * H + ho + 1) * DK]
                                # col-tiled PV, T0/T1 alternated so each
                                # adjacent pair runs concurrently
                                nc.tensor.matmul(out=cxq[0][0:64, :],
                                                 lhsT=vhe, rhs=eA[:, 0:512],
                                                 start=st, stop=sp_,
                                                 skip_group_check=True)
                                nc.tensor.matmul(out=cxq[0][64:128, :],
                                                 lhsT=vho, rhs=eA[:, 512:1024],
                                                 start=st, stop=sp_,
                                                 skip_group_check=True)
                                nc.tensor.matmul(out=cxq[1][0:64, :],
                                                 lhsT=vhe, rhs=eB[:, 0:512],
                                                 start=st, stop=sp_,
                                                 skip_group_check=True)
                                nc.tensor.matmul(out=cxq[1][64:128, :],
                                                 lhsT=vho, rhs=eB[:, 512:1024],
                                                 start=st, stop=sp_,
                                                 skip_group_check=True)
                            for scc in (g - 2, g - 1):
                                st = (scc == 0)
                                sp_ = (scc == 15)
                                eA = e_hist.pop((scc, 0))
                                eB = e_hist.pop((scc, 1))
                                # 4-way col-tiled denominators (M=32,
                                # all-ones lhsT: every row of the bank
                                # holds a valid denominator copy)
                                nc.tensor.matmul(out=den[0:32, :],
                                                 lhsT=onesm, rhs=eA[:, 0:512],
                                                 start=st, stop=sp_,
                                                 skip_group_check=True)
                                nc.tensor.matmul(out=den[32:64, :],
                                                 lhsT=onesm, rhs=eB[:, 0:512],
                                                 start=st, stop=sp_,
                                                 skip_group_check=True)
                                nc.tensor.matmul(out=den[64:96, :],
                                                 lhsT=onesm, rhs=eA[:, 512:1024],
                                                 start=st, stop=sp_,
                                                 skip_group_check=True)
                                nc.tensor.matmul(out=den[96:128, :],
                                                 lhsT=onesm, rhs=eB[:, 512:1024],
                                                 start=st, stop=sp_,
                                                 skip_group_check=True,
                                                 tile_position=(0, 96))
                            drive(filler, 2)

                    # drain ctx: copy PSUM->SBUF fast (frees cx + den),
                    # then normalize
                    st_t = stg.tile([128, 1024], dt.float32, tag="stg",
                                    name=f"stg{i}")
                    for q2 in range(2):
                        nc.vector.tensor_copy(
                            st_t[:, q2 * 512:(q2 + 1) * 512], cxq[q2])
                    # den bank fully valid (M=32 all-ones groups): one
                    # base-0 reciprocal_approx_fast over the den copy, DMA
                    # reciprocal rows to partition-0 tiles, baseline-style
                    # [64,512] broadcasts, 4 mixed-base muls.
                    dsb = rpool.tile([128, 512], dt.float32, tag="dsb",
                                     name=f"dsb{i}")
                    nc.vector.tensor_copy(dsb, den)
                    dsr = rpool.tile([128, 512], dt.float32, tag="dsr",
                                     name=f"dsr{i}")
                    nc.vector.reciprocal_approx_fast(out=dsr, in_=dsb)
                    rcs = []
                    for j in range(4):
                        rc = rpool.tile([1, 512], dt.float32, tag=f"rc{j}",
                                        name=f"rc{j}_{i}")
                        nc.sync.dma_start(out=rc, in_=dsr[32 * j:32 * j + 1, :])
                        rcs.append(rc)
                    rbs = []
                    for j in range(4):
                        rb = rpool.tile([128, 512], dt.float32, tag=f"rb{j}",
                                        name=f"rb{j}_{i}")
                        nc.gpsimd.partition_broadcast(rb, rcs[j][0:1, :])
                        rbs.append(rb)
                    # st_t cols 0:512=q0, 512:1024=q1; rows 0:64=he, 64:128=ho
                    # (tensor_tensor needs equal input base partitions, so
                    # rb tiles are full-height broadcasts)
                    nc.vector.tensor_mul(
                        out=ctxt_s[0:64, i * SQ:i * SQ + 512],
                        in0=st_t[0:64, 0:512], in1=rbs[0][0:64, :])
                    nc.vector.tensor_mul(
                        out=ctxt_s[0:64, i * SQ + 512:i * SQ + 1024],
                        in0=st_t[0:64, 512:1024], in1=rbs[1][0:64, :])
                    nc.vector.tensor_mul(
                        out=ctxt_s[64:128, i * SQ:i * SQ + 512],
                        in0=st_t[64:128, 0:512], in1=rbs[2][64:128, :])
                    nc.vector.tensor_mul(
                        out=ctxt_s[64:128, i * SQ + 512:i * SQ + 1024],
                        in0=st_t[64:128, 512:1024], in1=rbs[3][64:128, :])
                    finish(filler)

                # ---- phase 3: output projection -----------------------
                for qc in range(8):
                    pO = spp.tile([128, 1024], dt.float32, tag="sp",
                                  name=f"pO{qc}")
                    for i in range(8):
                        lhsT = ctxt_s[:, i * SQ + qc * 128: i * SQ + (qc + 1) * 128]
                        for do2 in range(2):
                            nc.tensor.matmul(
                                out=pO[:, do2 * 512:(do2 + 1) * 512],
                                lhsT=lhsT,
                                rhs=wo_s[:, i * D + do2 * 512: i * D + (do2 + 1) * 512],
                                start=(i == 0), stop=(i == 7))
                    ot = opool.tile([128, 1024], dt.float32, tag="ot",
                                    name=f"ot{qc}")
                    nc.vector.tensor_add(out=ot, in0=pO, in1=bob)
                    nc.sync.dma_start(
                        out=out[qc * 128:(qc + 1) * 128, :], in_=ot)

    nc.compile()
    return nc


def _make_in_maps(inputs):
    hidden_states = inputs["hidden_states"]
    attention_mask = inputs["attention_mask"]
    wq_b = np.ascontiguousarray(np.asarray(inputs["Wq"]).astype(BF))
    wk_b = np.ascontiguousarray(np.asarray(inputs["Wk"]).astype(BF))
    wv_b = np.ascontiguousarray(np.asarray(inputs["Wv"]).astype(BF))
    wo_b = np.ascontiguousarray(np.asarray(inputs["Wo"]).astype(BF))
    bq_f = np.ascontiguousarray(np.asarray(inputs["bq"]).astype(np.float32))
    bk_f = np.ascontiguousarray(np.asarray(inputs["bk"]).astype(np.float32))
    radd = (np.asarray(inputs["bv"]).astype(np.float32) @
            np.asarray(inputs["Wo"]).astype(np.float32) +
            np.asarray(inputs["bo"]).astype(np.float32))
    radd = np.ascontiguousarray(radd.astype(np.float32))

    in_maps = []
    for c in range(N_CORES):
        b, j = c // 2, c % 2
        xt_b = np.ascontiguousarray(np.asarray(hidden_states[b]).T.astype(BF))
        in_maps.append({
            "xt": xt_b,
            "xqt": np.ascontiguousarray(xt_b[:, j * SQ:(j + 1) * SQ]),
            "wq": wq_b, "wk": wk_b, "wv": wv_b, "wo": wo_b,
            "bq": bq_f, "bk": bk_f, "radd": radd,
            "mk": np.ascontiguousarray(
                np.asarray(attention_mask[b, 0, 0, :]).astype(np.float32) + NEG_C),
        })
    return in_maps


def kernel(hidden_states, attention_mask, Wq, bq, Wk, bk, Wv, bv, Wo, bo):
    from concourse.bass_utils import run_bass_kernel_spmd

    nc = _build()
    in_maps = _make_in_maps(dict(
        hidden_states=hidden_states, attention_mask=attention_mask,
        Wq=Wq, bq=bq, Wk=Wk, bk=bk, Wv=Wv, bv=bv, Wo=Wo, bo=bo))
    res = run_bass_kernel_spmd(nc, in_maps, list(range(N_CORES)))

    full = np.empty((B, S, D), dtype=np.float32)
    for c in range(N_CORES):
        b, j = c // 2, c % 2
        full[b, j * SQ:(j + 1) * SQ, :] = res.results[c]["out"]
    return full
